# revision 10
# baseline (speedup 1.0000x reference)
"""MoE transformer block on 8 NeuronCores (Bass/Tile).

Sharding:
  - Tokens: core c holds 256 tokens of batch 0 (rows c*256..) and 256 of
    batch 1 (rows c*256..) -> 512 tokens/core ("vt" order).
  - Attention: token-sharded fp32 QKV projections, AllToAll to head-sharded
    form (core c: batch c//4, heads (c%4)*4..+4, full causal, fp32),
    AllToAll back, Wo + residual + LN2 + router token-sharded (fp32).
  - MoE: expert-parallel (expert c on core c, bf16 FFN). Dispatch built with
    matmul prefix-sum compaction + indirect DMA; combine via ReduceScatter.

Outputs per core: x [512,1024] f32, probs [512,8] f32, ti [512,2] int32.
"""

import numpy as np
import ml_dtypes

import concourse.bass as bass
import concourse.bacc as bacc
import concourse.mybir as mybir
import concourse.tile as tile
from concourse.bass_utils import run_bass_kernel_spmd

F32 = mybir.dt.float32
F32R = mybir.dt.float32r
BF16 = mybir.dt.bfloat16
I32 = mybir.dt.int32
U32 = mybir.dt.uint32
U8 = mybir.dt.uint8
AF = mybir.ActivationFunctionType
OP = mybir.AluOpType

P = 128
NCORES = 8
B, S, D, H, E, K, F = 2, 2048, 1024, 16, 8, 2, 2048
HD = D // H          # 64
T = B * S            # 4096
TPC = T // NCORES    # 512
TB = 256             # tokens per (core, batch)
DC = D // P          # 8
FC = F // P          # 16
NT = TPC // P        # 4
HU = 4               # head units per core
NQ = S // P          # 16 q-chunks per unit
GCOL = T // P        # 32
CAP = 1152
NA = CAP // P        # 9
NEG = -1.0e30

GROUPS8 = [list(range(NCORES))]

_CACHE = {}


def _build():
    nc = bacc.Bacc("TRN2", target_bir_lowering=False, debug=False,
                   enable_asserts=True, num_devices=NCORES)

    def din(name, shape, dt=F32):
        return nc.dram_tensor(name, shape, dt, kind="ExternalInput")

    io = dict(
        xT_d=din("xT", [D, TPC]),
        wq_d=din("wq", [D, D]), wk_d=din("wk", [D, D]),
        wv_d=din("wv", [D, D]), wo_d=din("wo", [D, D]),
        wr_d=din("wr", [D, E]),
        ln1g_d=din("ln1g", [D, 1]), ln1b_d=din("ln1b", [D, 1]),
        ln2g_d=din("ln2g", [D, 1]), ln2b_d=din("ln2b", [D, 1]),
        bq_d=din("bq", [D, 1]), bk_d=din("bk", [D, 1]), bv_d=din("bv", [D, 1]),
        bo_d=din("bo", [1, D]), br_d=din("br", [1, E]),
        wg_d=din("wg", [D, F], BF16), wu_d=din("wu", [D, F], BF16),
        wd_d=din("wd", [F, D], BF16),
        bg_d=din("bg", [F, 1]), bu_d=din("bu", [F, 1]), bd_d=din("bd", [1, D]),
        cid_d=din("cid", [1, 1]),
        strictu_d=din("strictu", [P, P]),
        negmask_d=din("negmask", [P, P]),
        ident_d=din("ident", [P, P]),
        identbf_d=din("identbf", [P, P], BF16),
        tidf_d=din("tidf", [P, GCOL]),
        xout_d=nc.dram_tensor("xout", [TPC, D], F32, kind="ExternalOutput"),
        probs_d=nc.dram_tensor("probs", [TPC, E], F32, kind="ExternalOutput"),
        ti_d=nc.dram_tensor("ti", [TPC, K], I32, kind="ExternalOutput"),
        a2a1_in=nc.dram_tensor("a2a1_in", [NCORES * 3 * HU * HD, TB], F32),
        a2a1_out=nc.dram_tensor("a2a1_out", [NCORES * 3 * HU * HD, TB], F32),
        a2a2_in=nc.dram_tensor("a2a2_in", [NCORES * HU * HD, TB], F32),
        a2a2_out=nc.dram_tensor("a2a2_out", [NCORES * HU * HD, TB], F32),
        paq_in=nc.dram_tensor("paq_in", [TPC, 4], F32),
        paq_all=nc.dram_tensor("paq_all", [T, 4], F32, addr_space="Shared"),
        xn_in=nc.dram_tensor("xn_in", [TPC, D], BF16),
        xn_all=nc.dram_tensor("xn_all", [T, D], BF16, addr_space="Shared"),
        pairs_dram=nc.dram_tensor("pairs_dram", [CAP + P, 4], F32),
        out_dense=nc.dram_tensor("out_dense", [T + 1, D], BF16),
        rs_out=nc.dram_tensor("rs_outd", [TPC, D], BF16),
    )

    with tile.TileContext(nc) as tc:
        _emit(nc, tc, io)
    nc.compile()
    return nc


def _emit(nc, tc, io):
    g = io
    from contextlib import ExitStack
    ctx = ExitStack()
    with ctx:
        const = ctx.enter_context(tc.tile_pool(name="const", bufs=1))
        p_top = ctx.enter_context(tc.tile_pool(name="p_top", bufs=1))
        p_fin = ctx.enter_context(tc.tile_pool(name="p_fin", bufs=1))

        def rearr_col(d):  # [D,1] / [F,1] -> [P, chunks]
            return d[:, :].rearrange("(c p) o -> p (c o)", p=P)

        ident = const.tile([P, P], F32)
        nc.sync.dma_start(out=ident[:], in_=g["ident_d"][:, :])
        identbf = const.tile([P, P], BF16)
        nc.sync.dma_start(out=identbf[:], in_=g["identbf_d"][:, :])
        negmask = const.tile([P, P], F32)
        nc.sync.dma_start(out=negmask[:], in_=g["negmask_d"][:, :])
        strictu = const.tile([P, P], F32)
        nc.sync.dma_start(out=strictu[:], in_=g["strictu_d"][:, :])
        ones_col = const.tile([P, 1], F32)
        nc.vector.memset(ones_col[:], 1.0)
        ones_r128 = const.tile([1, P], F32)
        nc.vector.memset(ones_r128[:], 1.0)
        ones_r512 = const.tile([1, TPC], F32)
        nc.vector.memset(ones_r512[:], 1.0)
        ln1g = const.tile([P, DC], F32)
        nc.sync.dma_start(out=ln1g[:], in_=rearr_col(g["ln1g_d"]))
        ln1b = const.tile([P, DC], F32)
        nc.sync.dma_start(out=ln1b[:], in_=rearr_col(g["ln1b_d"]))
        ln2g = const.tile([P, DC], F32)
        nc.sync.dma_start(out=ln2g[:], in_=rearr_col(g["ln2g_d"]))
        ln2b = const.tile([P, DC], F32)
        nc.sync.dma_start(out=ln2b[:], in_=rearr_col(g["ln2b_d"]))
        bq_s = const.tile([P, DC], F32)
        nc.sync.dma_start(out=bq_s[:], in_=rearr_col(g["bq_d"]))
        bk_s = const.tile([P, DC], F32)
        nc.sync.dma_start(out=bk_s[:], in_=rearr_col(g["bk_d"]))
        bv_s = const.tile([P, DC], F32)
        nc.sync.dma_start(out=bv_s[:], in_=rearr_col(g["bv_d"]))
        bo_s = const.tile([1, D], F32)
        nc.sync.dma_start(out=bo_s[:], in_=g["bo_d"][:, :])
        br_s = const.tile([1, E], F32)
        nc.sync.dma_start(out=br_s[:], in_=g["br_d"][:, :])
        bg_s = const.tile([P, FC], F32)
        nc.sync.dma_start(out=bg_s[:], in_=rearr_col(g["bg_d"]))
        bu_s = const.tile([P, FC], F32)
        nc.sync.dma_start(out=bu_s[:], in_=rearr_col(g["bu_d"]))
        bd_s = const.tile([1, D], F32)
        nc.sync.dma_start(out=bd_s[:], in_=g["bd_d"][:, :])
        cid_b = const.tile([P, 1], F32)
        nc.sync.dma_start(out=cid_b[:], in_=g["cid_d"][0:1, :].partition_broadcast(P))
        tidf = const.tile([P, GCOL], F32)
        nc.sync.dma_start(out=tidf[:], in_=g["tidf_d"][:, :])
        eps_t = const.tile([1, 1], F32)
        nc.vector.memset(eps_t[:], 1e-5)

        # early zero-fills
        zero_bf = const.tile([P, D], BF16)
        nc.vector.memset(zero_bf[:], 0.0)
        for i in range(T // P):
            nc.sync.dma_start(out=g["out_dense"][i * P:(i + 1) * P, :], in_=zero_bf[:])
        nc.sync.dma_start(out=g["out_dense"][T:T + 1, :], in_=zero_bf[0:1, :])
        zero4 = const.tile([P, 4], F32)
        nc.vector.memset(zero4[:], 0.0)
        for a in range(NA + 1):
            nc.sync.dma_start(out=g["pairs_dram"][a * P:(a + 1) * P, :], in_=zero4[:])

        xT = p_top.tile([P, DC, TPC], F32)
        for dc in range(DC):
            nc.sync.dma_start(out=xT[:, dc, :], in_=g["xT_d"][dc * P:(dc + 1) * P, :])

        # ---------------- LayerNorm helper ----------------
        def layernorm(src, g_ap, b_ap, dst, work, lnps, extra_bf=None):
            sum_ps = lnps.tile([1, TPC], F32, tag="ln_sum")
            ssq_ps = lnps.tile([1, TPC], F32, tag="ln_ssq")
            for dc in range(DC):
                nc.tensor.matmul(out=sum_ps[:], lhsT=ones_col[:], rhs=src[:, dc, :],
                                 start=(dc == 0), stop=(dc == DC - 1))
            for dc in range(DC):
                sq = work.tile([P, TPC], F32, tag="ln_sq")
                nc.vector.tensor_tensor(out=sq[:], in0=src[:, dc, :],
                                        in1=src[:, dc, :], op=OP.mult)
                nc.tensor.matmul(out=ssq_ps[:], lhsT=ones_col[:], rhs=sq[:],
                                 start=(dc == 0), stop=(dc == DC - 1))
            mean = work.tile([1, TPC], F32, tag="ln_mean")
            nc.scalar.mul(out=mean[:], in_=sum_ps[:], mul=1.0 / D)
            msq = work.tile([1, TPC], F32, tag="ln_msq")
            nc.scalar.mul(out=msq[:], in_=ssq_ps[:], mul=1.0 / D)
            var = work.tile([1, TPC], F32, tag="ln_var")
            nc.vector.tensor_tensor(out=var[:], in0=mean[:], in1=mean[:], op=OP.mult)
            nc.vector.tensor_sub(out=var[:], in0=msq[:], in1=var[:])
            std = work.tile([1, TPC], F32, tag="ln_std")
            nc.scalar.activation(out=std[:], in_=var[:], func=AF.Sqrt, bias=eps_t[:])
            rstd = work.tile([1, TPC], F32, tag="ln_rstd")
            nc.vector.reciprocal(out=rstd[:], in_=std[:])
            mean_b_ps = lnps.tile([P, TPC], F32, tag="ln_bc")
            nc.tensor.matmul(out=mean_b_ps[:], lhsT=ones_r128[:], rhs=mean[:],
                             start=True, stop=True)
            mean_b = work.tile([P, TPC], F32, tag="ln_meanb")
            nc.vector.tensor_copy(out=mean_b[:], in_=mean_b_ps[:])
            rstd_b_ps = lnps.tile([P, TPC], F32, tag="ln_bc")
            nc.tensor.matmul(out=rstd_b_ps[:], lhsT=ones_r128[:], rhs=rstd[:],
                             start=True, stop=True)
            rstd_b = work.tile([P, TPC], F32, tag="ln_rstdb")
            nc.vector.tensor_copy(out=rstd_b[:], in_=rstd_b_ps[:])
            for dc in range(DC):
                tmp = work.tile([P, TPC], F32, tag="ln_tmp")
                nc.vector.tensor_sub(out=tmp[:], in0=src[:, dc, :], in1=mean_b[:])
                nc.vector.tensor_tensor(out=tmp[:], in0=tmp[:], in1=rstd_b[:],
                                        op=OP.mult)
                nc.scalar.activation(out=dst[:, dc, :], in_=tmp[:], func=AF.Identity,
                                     bias=b_ap[:, dc:dc + 1], scale=g_ap[:, dc:dc + 1])
                if extra_bf is not None:
                    nc.vector.tensor_copy(out=extra_bf[:, dc, :], in_=dst[:, dc, :])

        stage1 = ctx.enter_context(tc.tile_pool(name="stage1", bufs=1))
        h1n = stage1.tile([P, DC, TPC], F32)
        with tc.tile_pool(name="ln1_work", bufs=3) as ln1_work, \
             tc.tile_pool(name="ln1_ps", bufs=1, space="PSUM") as ln1_ps:
            layernorm(xT, ln1g, ln1b, h1n, ln1_work, ln1_ps)

        # ---------------- QKV projections (fp32) -> a2a1_in ----------------
        with tc.tile_pool(name="proj", bufs=4) as proj, \
             tc.tile_pool(name="proj_ps", bufs=2, space="PSUM") as proj_ps:
            for wi, (w_d, b_s, base) in enumerate((
                    (g["wq_d"], bq_s, 0),
                    (g["wk_d"], bk_s, HU * HD),
                    (g["wv_d"], bv_s, 2 * HU * HD))):
                for mc in range(DC):
                    acc = proj_ps.tile([P, TPC], F32, tag="proj_acc")
                    for dc in range(DC):
                        wt = proj.tile([P, P], F32, tag="w")
                        nc.sync.dma_start(
                            out=wt[:],
                            in_=w_d[dc * P:(dc + 1) * P, mc * P:(mc + 1) * P])
                        nc.tensor.matmul(out=acc[:], lhsT=wt[:], rhs=h1n[:, dc, :],
                                         start=(dc == 0), stop=(dc == DC - 1))
                    ev = proj.tile([P, TPC], F32, tag="ev")
                    nc.scalar.activation(out=ev[:], in_=acc[:], func=AF.Identity,
                                         bias=b_s[:, mc:mc + 1])
                    # ship: dest d0 = mc//2 (batch0 cols) and d1 = 4 + mc//2
                    half = (mc % 2) * P
                    d0 = mc // 2
                    d1 = 4 + mc // 2
                    nc.sync.dma_start(
                        out=g["a2a1_in"][d0 * 768 + base + half:d0 * 768 + base + half + P, :],
                        in_=ev[:, 0:TB])
                    nc.sync.dma_start(
                        out=g["a2a1_in"][d1 * 768 + base + half:d1 * 768 + base + half + P, :],
                        in_=ev[:, TB:2 * TB])

        nc.gpsimd.collective_compute(
            "AllToAll", OP.bypass, replica_groups=GROUPS8,
            ins=[g["a2a1_in"][:, :]], outs=[g["a2a1_out"][:, :]])

        # ---------------- attention: 4 units, full causal ----------------
        ao_scope = tc.tile_pool(name="p_ao", bufs=1)
        p_ao = ao_scope.__enter__()
        aoTn = p_ao.tile([64, HU, S], F32)
        with tc.tile_pool(name="attn", bufs=2) as attn, \
             tc.tile_pool(name="attn_ps", bufs=1, space="PSUM") as attn_ps, \
             tc.tile_pool(name="attn_ps2", bufs=2, space="PSUM") as attn_ps2:
            for u in range(HU):
                qt = attn.tile([64, S], F32, tag="qt")
                kt = attn.tile([64, S], F32, tag="kt")
                vtok = attn.tile([P, NQ, 64], F32, tag="vtok")
                for i in range(NCORES):
                    r0 = i * 768 + u * 64
                    nc.sync.dma_start(out=qt[:, i * TB:(i + 1) * TB],
                                      in_=g["a2a1_out"][r0:r0 + 64, :])
                    nc.sync.dma_start(out=kt[:, i * TB:(i + 1) * TB],
                                      in_=g["a2a1_out"][r0 + 256:r0 + 256 + 64, :])
                    vt_tmp = attn.tile([64, TB], F32, tag="vt_tmp")
                    nc.sync.dma_start(out=vt_tmp[:],
                                      in_=g["a2a1_out"][r0 + 512:r0 + 512 + 64, :])
                    for j in range(TB // P):
                        vps = attn_ps2.tile([P, 64], F32, tag="pt")
                        nc.tensor.transpose(out=vps[:], in_=vt_tmp[:, j * P:(j + 1) * P],
                                            identity=ident[0:64, 0:64])
                        nc.vector.tensor_copy(out=vtok[:, i * 2 + j, :], in_=vps[:])

                qt_r = attn.tile([64, S], F32R, tag="qt_r")
                nc.vector.tensor_copy(out=qt_r[:], in_=qt[:])
                kt_r = attn.tile([64, S], F32R, tag="kt_r")
                nc.vector.tensor_copy(out=kt_r[:], in_=kt[:])
                for qi in range(NQ):
                    lk = (qi + 1) * P
                    p_sb = attn.tile([P, S], F32, tag="p_sb")
                    rs_tot = attn.tile([P, 1], F32, tag="rs_tot")
                    nhalf = (lk + 1023) // 1024
                    for hh in range(nhalf):
                        h0 = hh * 1024
                        hw = min(1024, lk - h0)
                        sc = attn_ps2.tile([P, 1024], F32, tag="sc")
                        for n0 in range(0, hw, 512):
                            nw = min(512, hw - n0)
                            nc.tensor.matmul(out=sc[:, n0:n0 + nw],
                                             lhsT=qt_r[:, qi * P:(qi + 1) * P],
                                             rhs=kt_r[:, h0 + n0:h0 + n0 + nw],
                                             start=True, stop=True)
                        if h0 + hw == lk:
                            nc.vector.tensor_add(out=sc[:, hw - P:hw],
                                                 in0=sc[:, hw - P:hw], in1=negmask[:])
                        rs_h = attn.tile([P, 1], F32, tag="rs_h")
                        nc.scalar.activation(out=p_sb[:, h0:h0 + hw], in_=sc[:, 0:hw],
                                             func=AF.Exp, scale=1.0 / 8.0,
                                             accum_out=rs_h[:])
                        if hh == 0:
                            nc.vector.tensor_copy(out=rs_tot[:], in_=rs_h[:])
                        else:
                            nc.vector.tensor_add(out=rs_tot[:], in0=rs_tot[:],
                                                 in1=rs_h[:])
                    rec = attn.tile([P, 1], F32, tag="rec")
                    nc.vector.reciprocal(out=rec[:], in_=rs_tot[:])
                    recT_ps = attn_ps.tile([1, P], F32, tag="rec")
                    nc.tensor.transpose(out=recT_ps[:], in_=rec[:], identity=ident[:])
                    recT = attn.tile([1, P], F32, tag="recT_sb")
                    nc.vector.tensor_copy(out=recT[:], in_=recT_ps[:])
                    recb_ps = attn_ps.tile([64, P], F32, tag="rec")
                    nc.tensor.matmul(out=recb_ps[:], lhsT=ones_r128[:, 0:64],
                                     rhs=recT[:], start=True, stop=True)
                    recb = attn.tile([64, P], F32, tag="recb_sb")
                    nc.vector.tensor_copy(out=recb[:], in_=recb_ps[:])
                    ao_ps = attn_ps.tile([64, P], F32, tag="ao")
                    for kc in range(qi + 1):
                        pt_ps = attn_ps2.tile([P, P], F32, tag="pt")
                        nc.tensor.transpose(out=pt_ps[:],
                                            in_=p_sb[:, kc * P:(kc + 1) * P],
                                            identity=ident[:])
                        pt_sb = attn.tile([P, P], F32, tag="pt_sb")
                        nc.vector.tensor_copy(out=pt_sb[:], in_=pt_ps[:])
                        nc.tensor.matmul(out=ao_ps[:], lhsT=vtok[:, kc, :],
                                         rhs=pt_sb[:],
                                         start=(kc == 0), stop=(kc == qi),
                                         skip_group_check=True)
                    nc.vector.tensor_tensor(out=aoTn[:, u, qi * P:(qi + 1) * P],
                                            in0=ao_ps[:], in1=recb[:], op=OP.mult)

        for u in range(HU):
            for d in range(NCORES):
                nc.sync.dma_start(
                    out=g["a2a2_in"][d * 256 + u * 64:d * 256 + (u + 1) * 64, :],
                    in_=aoTn[:, u, d * TB:(d + 1) * TB])
        ao_scope.__exit__(None, None, None)

        nc.gpsimd.collective_compute(
            "AllToAll", OP.bypass, replica_groups=GROUPS8,
            ins=[g["a2a2_in"][:, :]], outs=[g["a2a2_out"][:, :]])

        # ---------------- Wo + residual + LN2 + router ----------------
        xmid_tok = p_fin.tile([P, NT, D], F32)
        wgt_sb = p_fin.tile([P, NA, 4], F32)
        outid_i = p_fin.tile([P, NA], I32)
        gidx_i = p_fin.tile([P, NA], I32)

        with tc.tile_pool(name="p_mid", bufs=1) as p_mid, \
             tc.tile_pool(name="mid", bufs=2) as mid, \
             tc.tile_pool(name="mid_ps", bufs=2, space="PSUM") as mid_ps, \
             tc.tile_pool(name="mid_ps1", bufs=1, space="PSUM") as mid_ps1:
            xmid = p_mid.tile([P, DC, TPC], F32)
            xn_f = p_mid.tile([P, DC, TPC], F32)
            xn_bf = p_mid.tile([P, DC, TPC], BF16)
            for mc in range(DC):
                acc = mid_ps.tile([P, TPC], F32, tag="wo_acc")
                nc.tensor.matmul(out=acc[:], lhsT=bo_s[:, mc * P:(mc + 1) * P],
                                 rhs=ones_r512[:], start=True, stop=False)
                for dc in range(DC):
                    aot = mid.tile([P, TPC], F32, tag="aot")
                    r_b0 = (dc // 2) * 256 + (dc % 2) * P
                    r_b1 = (4 + dc // 2) * 256 + (dc % 2) * P
                    nc.sync.dma_start(out=aot[:, 0:TB],
                                      in_=g["a2a2_out"][r_b0:r_b0 + P, :])
                    nc.sync.dma_start(out=aot[:, TB:2 * TB],
                                      in_=g["a2a2_out"][r_b1:r_b1 + P, :])
                    wt = mid.tile([P, P], F32, tag="wo_w")
                    nc.sync.dma_start(
                        out=wt[:],
                        in_=g["wo_d"][dc * P:(dc + 1) * P, mc * P:(mc + 1) * P])
                    nc.tensor.matmul(out=acc[:], lhsT=wt[:], rhs=aot[:],
                                     start=False, stop=(dc == DC - 1))
                nc.vector.tensor_tensor(out=xmid[:, mc, :], in0=acc[:],
                                        in1=xT[:, mc, :], op=OP.add)

            for mc in range(DC):
                for tcn in range(NT):
                    tp = mid_ps.tile([P, P], F32, tag="tp")
                    nc.tensor.transpose(out=tp[:],
                                        in_=xmid[:, mc, tcn * P:(tcn + 1) * P],
                                        identity=ident[:])
                    nc.vector.tensor_copy(out=xmid_tok[:, tcn, mc * P:(mc + 1) * P],
                                          in_=tp[:])

            layernorm(xmid, ln2g, ln2b, xn_f, mid, mid_ps1, extra_bf=xn_bf)

            lg_ps = mid_ps1.tile([E, TPC], F32, tag="lg")
            nc.tensor.matmul(out=lg_ps[:], lhsT=br_s[:], rhs=ones_r512[:],
                             start=True, stop=False)
            for dc in range(DC):
                wrt = mid.tile([P, E], F32, tag="wr")
                nc.sync.dma_start(out=wrt[:], in_=g["wr_d"][dc * P:(dc + 1) * P, :])
                nc.tensor.matmul(out=lg_ps[:], lhsT=wrt[:], rhs=xn_f[:, dc, :],
                                 start=False, stop=(dc == DC - 1))
            lg_sb = mid.tile([E, TPC], F32, tag="lg_sb")
            nc.vector.tensor_copy(out=lg_sb[:], in_=lg_ps[:])

            paq_sb = mid.tile([P, NT, 4], F32, tag="paq")
            for tcn in range(NT):
                lt_ps = mid_ps.tile([P, P], F32, tag="tp")
                nc.tensor.transpose(out=lt_ps[:, 0:E],
                                    in_=lg_sb[:, tcn * P:(tcn + 1) * P],
                                    identity=ident[0:E, 0:E])
                lt = mid.tile([P, E], F32, tag="lt_sb")
                nc.vector.tensor_copy(out=lt[:], in_=lt_ps[:, 0:E])
                mx = mid.tile([P, 8], F32, tag="mx")
                mi = mid.tile([P, 8], U32, tag="mi")
                nc.vector.max_with_indices(mx[:], mi[:], lt[:])
                ti_sb = mid.tile([P, K], I32, tag="ti_sb")
                nc.vector.tensor_copy(out=ti_sb[:], in_=mi[:, 0:K].bitcast(I32))
                nc.sync.dma_start(out=g["ti_d"][tcn * P:(tcn + 1) * P, :], in_=ti_sb[:])
                negmx = mid.tile([P, 1], F32, tag="negmx")
                nc.vector.tensor_scalar_mul(negmx[:], mx[:, 0:1], -1.0)
                e8 = mid.tile([P, E], F32, tag="e8")
                s8 = mid.tile([P, 1], F32, tag="s8")
                nc.scalar.activation(out=e8[:], in_=lt[:], func=AF.Exp,
                                     bias=negmx[:], accum_out=s8[:])
                r8 = mid.tile([P, 1], F32, tag="r8")
                nc.vector.reciprocal(out=r8[:], in_=s8[:])
                pr = mid.tile([P, E], F32, tag="pr")
                nc.vector.tensor_scalar(out=pr[:], in0=e8[:], scalar1=r8[:],
                                        scalar2=None, op0=OP.mult)
                nc.sync.dma_start(out=g["probs_d"][tcn * P:(tcn + 1) * P, :], in_=pr[:])
                dif = mid.tile([P, 1], F32, tag="dif")
                nc.vector.tensor_sub(out=dif[:], in0=mx[:, 0:1], in1=mx[:, 1:2])
                tp0 = mid.tile([P, 1], F32, tag="tp0")
                nc.scalar.activation(out=tp0[:], in_=dif[:], func=AF.Sigmoid)
                nc.vector.tensor_copy(out=paq_sb[:, tcn, 0:1], in_=tp0[:])
                nc.vector.tensor_scalar(out=paq_sb[:, tcn, 1:2], in0=tp0[:],
                                        scalar1=-1.0, scalar2=1.0,
                                        op0=OP.mult, op1=OP.add)
                nc.vector.tensor_copy(out=paq_sb[:, tcn, 2:3], in_=mi[:, 0:1])
                nc.vector.tensor_copy(out=paq_sb[:, tcn, 3:4], in_=mi[:, 1:2])
            nc.sync.dma_start(
                out=g["paq_in"][:, :].rearrange("(c p) k -> p c k", p=P),
                in_=paq_sb[:])

            xn_tok = mid.tile([P, NT, D], BF16, tag="xn_tok")
            for dc in range(DC):
                for tcn in range(NT):
                    tp = mid_ps.tile([P, P], BF16, tag="tp")
                    nc.tensor.transpose(out=tp[:],
                                        in_=xn_bf[:, dc, tcn * P:(tcn + 1) * P],
                                        identity=identbf[:])
                    nc.vector.tensor_copy(out=xn_tok[:, tcn, dc * P:(dc + 1) * P],
                                          in_=tp[:])
            for tcn in range(NT):
                nc.sync.dma_start(out=g["xn_in"][tcn * P:(tcn + 1) * P, :],
                                  in_=xn_tok[:, tcn, :])

        nc.gpsimd.collective_compute(
            "AllGather", OP.bypass, replica_groups=GROUPS8,
            ins=[g["paq_in"][:, :]], outs=[g["paq_all"][:, :]])
        nc.gpsimd.collective_compute(
            "AllGather", OP.bypass, replica_groups=GROUPS8,
            ins=[g["xn_in"][:, :]], outs=[g["xn_all"][:, :]])

        # ---------------- dispatch ----------------
        with tc.tile_pool(name="disp", bufs=2) as disp, \
             tc.tile_pool(name="disp_ps", bufs=2, space="PSUM") as disp_ps:
            paq_t = disp.tile([P, GCOL, 4], F32, tag="paq_t")
            nc.sync.dma_start(out=paq_t[:],
                              in_=g["paq_all"][:, :].rearrange("(c p) k -> p c k", p=P))
            m0 = disp.tile([P, GCOL], F32, tag="m0")
            nc.vector.tensor_tensor(out=m0[:], in0=paq_t[:, :, 2],
                                    in1=cid_b[:].to_broadcast([P, GCOL]),
                                    op=OP.is_equal)
            m1 = disp.tile([P, GCOL], F32, tag="m1")
            nc.vector.tensor_tensor(out=m1[:], in0=paq_t[:, :, 3],
                                    in1=cid_b[:].to_broadcast([P, GCOL]),
                                    op=OP.is_equal)
            maskf = disp.tile([P, GCOL], F32, tag="maskf")
            nc.vector.tensor_add(out=maskf[:], in0=m0[:], in1=m1[:])
            mask8 = disp.tile([P, GCOL], U8, tag="mask8")
            nc.vector.tensor_copy(out=mask8[:], in_=maskf[:])
            wsel = disp.tile([P, GCOL], F32, tag="wsel")
            nc.vector.tensor_tensor(out=m0[:], in0=m0[:], in1=paq_t[:, :, 0], op=OP.mult)
            nc.vector.tensor_tensor(out=m1[:], in0=m1[:], in1=paq_t[:, :, 1], op=OP.mult)
            nc.vector.tensor_add(out=wsel[:], in0=m0[:], in1=m1[:])

            cs_ps = disp_ps.tile([1, GCOL], F32, tag="cs")
            nc.tensor.matmul(out=cs_ps[:], lhsT=ones_col[:], rhs=maskf[:],
                             start=True, stop=True)
            pfx_a = disp.tile([1, GCOL], F32, tag="pfx_a")
            pfx_b = disp.tile([1, GCOL], F32, tag="pfx_b")
            nc.vector.memset(pfx_a[:], 0.0)
            nc.vector.tensor_copy(out=pfx_a[:, 1:GCOL], in_=cs_ps[0:1, 0:GCOL - 1])
            cur, oth = pfx_a, pfx_b
            for sh in (1, 2, 4, 8, 16):
                nc.vector.tensor_copy(out=oth[:], in_=cur[:])
                nc.vector.tensor_add(out=oth[:, sh:GCOL], in0=cur[:, sh:GCOL],
                                     in1=cur[:, 0:GCOL - sh])
                cur, oth = oth, cur
            p_ps = disp_ps.tile([P, GCOL], F32, tag="p_ps")
            nc.tensor.matmul(out=p_ps[:], lhsT=strictu[:], rhs=maskf[:],
                             start=True, stop=False)
            nc.tensor.matmul(out=p_ps[:], lhsT=ones_r128[:], rhs=cur[:],
                             start=False, stop=True)
            trash = disp.tile([P, GCOL], F32, tag="trash")
            nc.vector.memset(trash[:], float(CAP))
            p_sb2 = disp.tile([P, GCOL], F32, tag="p_sb2")
            nc.vector.tensor_copy(out=p_sb2[:], in_=p_ps[:])
            pos_f = disp.tile([P, GCOL], F32, tag="pos_f")
            nc.vector.select(out=pos_f[:], mask=mask8[:], on_true=p_sb2[:],
                             on_false=trash[:])
            nc.vector.tensor_scalar_min(pos_f[:], pos_f[:], float(CAP))
            pos_i = disp.tile([P, GCOL], I32, tag="pos_i")
            nc.vector.tensor_copy(out=pos_i[:], in_=pos_f[:])

            vals4 = disp.tile([P, GCOL, 4], F32, tag="vals4")
            nc.vector.tensor_copy(out=vals4[:, :, 0], in_=tidf[:])
            nc.vector.tensor_copy(out=vals4[:, :, 1], in_=wsel[:])
            nc.vector.memset(vals4[:, :, 2], 1.0)
            nc.vector.memset(vals4[:, :, 3], 0.0)
            for col in range(GCOL):
                nc.gpsimd.indirect_dma_start(
                    out=g["pairs_dram"][:, :],
                    out_offset=bass.IndirectOffsetOnAxis(
                        ap=pos_i[:, col:col + 1], axis=0),
                    in_=vals4[:, col, :], in_offset=None)

            nc.sync.dma_start(
                out=wgt_sb[:],
                in_=g["pairs_dram"][0:CAP, :].rearrange("(a p) k -> p a k", p=P))
            nc.vector.tensor_copy(out=gidx_i[:], in_=wgt_sb[:, :, 0])
            valid8 = disp.tile([P, NA], U8, tag="valid8")
            nc.vector.tensor_copy(out=valid8[:], in_=wgt_sb[:, :, 2])
            t4096 = disp.tile([P, NA], F32, tag="t4096")
            nc.vector.memset(t4096[:], float(T))
            outid_f = disp.tile([P, NA], F32, tag="outid_f")
            nc.vector.select(out=outid_f[:], mask=valid8[:], on_true=wgt_sb[:, :, 0],
                             on_false=t4096[:])
            nc.vector.tensor_copy(out=outid_i[:], in_=outid_f[:])

        # ---------------- expert FFN (bf16) ----------------
        with tc.tile_pool(name="ffn", bufs=2) as ffn, \
             tc.tile_pool(name="ffn_w", bufs=2) as ffn_w, \
             tc.tile_pool(name="ffn_big", bufs=1) as ffn_big, \
             tc.tile_pool(name="ffn_ps", bufs=2, space="PSUM") as ffn_ps:
            xgT = ffn_big.tile([P, DC, CAP], BF16)
            with tc.tile_pool(name="xg_pool", bufs=1) as xgp:
                xg_tok = xgp.tile([P, NA, D], BF16)
                for a in range(NA):
                    nc.gpsimd.indirect_dma_start(
                        out=xg_tok[:, a, :], out_offset=None,
                        in_=g["xn_all"][:, :],
                        in_offset=bass.IndirectOffsetOnAxis(ap=gidx_i[:, a:a + 1], axis=0))
                for a in range(NA):
                    for dc in range(DC):
                        tp = ffn_ps.tile([P, P], BF16, tag="xg_t")
                        nc.tensor.transpose(out=tp[:],
                                            in_=xg_tok[:, a, dc * P:(dc + 1) * P],
                                            identity=identbf[:])
                        nc.vector.tensor_copy(out=xgT[:, dc, a * P:(a + 1) * P], in_=tp[:])

            hg = ffn_big.tile([P, FC, CAP], BF16)
            NCH = tuple((n0, min(512, CAP - n0)) for n0 in range(0, CAP, 512))
            for fc in range(FC):
                wgt_f = ffn_w.tile([P, DC, P], BF16, tag="wg_f")
                nc.sync.dma_start(
                    out=wgt_f[:],
                    in_=g["wg_d"][:, fc * P:(fc + 1) * P].rearrange(
                        "(dc p) f -> p dc f", p=P))
                wut_f = ffn_w.tile([P, DC, P], BF16, tag="wu_f")
                nc.sync.dma_start(
                    out=wut_f[:],
                    in_=g["wu_d"][:, fc * P:(fc + 1) * P].rearrange(
                        "(dc p) f -> p dc f", p=P))
                for (n0, nw) in NCH:
                    g_ps = ffn_ps.tile([P, 512], F32, tag="g_ps")
                    u_ps = ffn_ps.tile([P, 512], F32, tag="u_ps")
                    for dc in range(DC):
                        nc.tensor.matmul(out=g_ps[:, 0:nw], lhsT=wgt_f[:, dc, :],
                                         rhs=xgT[:, dc, n0:n0 + nw],
                                         start=(dc == 0), stop=(dc == DC - 1))
                    for dc in range(DC):
                        nc.tensor.matmul(out=u_ps[:, 0:nw], lhsT=wut_f[:, dc, :],
                                         rhs=xgT[:, dc, n0:n0 + nw],
                                         start=(dc == 0), stop=(dc == DC - 1))
                    g_sb = ffn.tile([P, 512], BF16, tag="g_sb")
                    nc.scalar.activation(out=g_sb[:, 0:nw], in_=g_ps[:, 0:nw],
                                         func=AF.Silu, bias=bg_s[:, fc:fc + 1])
                    nc.vector.scalar_tensor_tensor(
                        out=hg[:, fc, n0:n0 + nw], in0=u_ps[:, 0:nw],
                        scalar=bu_s[:, fc:fc + 1], in1=g_sb[:, 0:nw],
                        op0=OP.add, op1=OP.mult)

            wd_sb = ffn_big.tile([P, FC, D], BF16)
            nc.sync.dma_start(out=wd_sb[:],
                              in_=g["wd_d"][:, :].rearrange("(fc p) d -> p fc d", p=P))
            ffn_out = ffn_big.tile([P, NA, D], BF16)
            for a in range(NA):
                for (n0, nw) in ((0, 512), (512, 512)):
                    o_ps = ffn_ps.tile([P, 512], F32, tag="o_ps")
                    nc.tensor.matmul(out=o_ps[:, 0:nw], lhsT=ones_r128[:],
                                     rhs=bd_s[:, n0:n0 + nw], start=True, stop=False)
                    for fc in range(FC):
                        nc.tensor.matmul(out=o_ps[:, 0:nw],
                                         lhsT=hg[:, fc, a * P:(a + 1) * P],
                                         rhs=wd_sb[:, fc, n0:n0 + nw],
                                         start=False, stop=(fc == FC - 1))
                    nc.scalar.mul(out=ffn_out[:, a, n0:n0 + nw], in_=o_ps[:, 0:nw],
                                  mul=wgt_sb[:, a, 1:2])
            for a in range(NA):
                nc.gpsimd.indirect_dma_start(
                    out=g["out_dense"][:, :],
                    out_offset=bass.IndirectOffsetOnAxis(
                        ap=outid_i[:, a:a + 1], axis=0),
                    in_=ffn_out[:, a, :], in_offset=None)

        nc.gpsimd.collective_compute(
            "ReduceScatter", OP.add, replica_groups=GROUPS8,
            ins=[g["out_dense"][0:T, :]], outs=[g["rs_out"][:, :]])
        with tc.tile_pool(name="fin", bufs=2) as fin:
            for tcn in range(NT):
                rs_sb = fin.tile([P, D], BF16, tag="rs_sb")
                nc.sync.dma_start(out=rs_sb[:],
                                  in_=g["rs_out"][tcn * P:(tcn + 1) * P, :])
                xo = fin.tile([P, D], F32, tag="xo")
                nc.vector.tensor_tensor(out=xo[:], in0=xmid_tok[:, tcn, :],
                                        in1=rs_sb[:], op=OP.add)
                nc.sync.dma_start(out=g["xout_d"][tcn * P:(tcn + 1) * P, :], in_=xo[:])


def _vt2flat():
    """vt -> row index into x.reshape(T, D); vt = c*512 + j."""
    idx = np.zeros(T, np.int64)
    for c in range(NCORES):
        j = np.arange(TPC)
        b = (j >= TB).astype(np.int64)
        s = c * TB + np.where(j < TB, j, j - TB)
        idx[c * TPC:(c + 1) * TPC] = b * S + s
    return idx


def _prep_inputs(inputs):
    f32 = lambda a: np.ascontiguousarray(np.asarray(a, dtype=np.float32))
    bf = lambda a: np.ascontiguousarray(
        np.asarray(a, dtype=np.float32).astype(ml_dtypes.bfloat16))

    x = f32(inputs["x"])
    flat = x.reshape(T, D)
    vt = _vt2flat()

    Wg = np.asarray(inputs["Wg"], dtype=np.float32)
    Wu = np.asarray(inputs["Wu"], dtype=np.float32)
    Wd = np.asarray(inputs["Wd"], dtype=np.float32)
    bg = np.asarray(inputs["bg"], dtype=np.float32)
    bu = np.asarray(inputs["bu"], dtype=np.float32)
    bd = np.asarray(inputs["bd"], dtype=np.float32)

    strictu = np.triu(np.ones((P, P), np.float32), 1)
    negmask = np.triu(np.full((P, P), NEG, np.float32), 1)
    ident = np.eye(P, dtype=np.float32)
    identbf = np.eye(P, dtype=np.float32).astype(ml_dtypes.bfloat16)
    tidf = (np.arange(GCOL)[None, :] * P + np.arange(P)[:, None]).astype(np.float32)

    common = dict(
        wq=f32(inputs["Wq"]), wk=f32(inputs["Wk"]), wv=f32(inputs["Wv"]),
        wo=f32(inputs["Wo"]), wr=f32(inputs["Wr"]),
        ln1g=f32(inputs["ln1_g"]).reshape(D, 1),
        ln1b=f32(inputs["ln1_b"]).reshape(D, 1),
        ln2g=f32(inputs["ln2_g"]).reshape(D, 1),
        ln2b=f32(inputs["ln2_b"]).reshape(D, 1),
        bq=f32(inputs["bq"]).reshape(D, 1), bk=f32(inputs["bk"]).reshape(D, 1),
        bv=f32(inputs["bv"]).reshape(D, 1), bo=f32(inputs["bo"]).reshape(1, D),
        br=f32(inputs["br"]).reshape(1, E),
        strictu=strictu, negmask=negmask, ident=ident, identbf=identbf,
        tidf=tidf,
    )
    in_maps = []
    for c in range(NCORES):
        m = dict(common)
        m["xT"] = np.ascontiguousarray(flat[vt[c * TPC:(c + 1) * TPC]].T)
        m["wg"] = bf(Wg[c])
        m["wu"] = bf(Wu[c])
        m["wd"] = bf(Wd[c])
        m["bg"] = f32(bg[c]).reshape(F, 1)
        m["bu"] = f32(bu[c]).reshape(F, 1)
        m["bd"] = f32(bd[c]).reshape(1, D)
        m["cid"] = np.array([[float(c)]], np.float32)
        in_maps.append(m)
    return in_maps


def kernel(**inputs):
    if "nc" not in _CACHE:
        _CACHE["nc"] = _build()
    nc = _CACHE["nc"]
    in_maps = _prep_inputs(inputs)
    res = run_bass_kernel_spmd(nc, in_maps, list(range(NCORES)))
    vt = _vt2flat()
    x = np.zeros((T, D), np.float32)
    probs = np.zeros((T, E), np.float32)
    ti = np.zeros((T, K), np.int32)
    for c in range(NCORES):
        r = res.results[c]
        sl = vt[c * TPC:(c + 1) * TPC]
        x[sl] = r["xout"]
        probs[sl] = r["probs"]
        ti[sl] = r["ti"]
    return (x.reshape(B, S, D), probs.reshape(B, S, E), ti.reshape(B, S, K))


# revision 11
# speedup vs baseline: 1.0725x; 1.0725x over previous
"""MoE transformer block on 8 NeuronCores (Bass/Tile).

Sharding:
  - Tokens: core c holds 256 tokens of batch 0 (rows c*256..) and 256 of
    batch 1 (rows c*256..) -> 512 tokens/core ("vt" order).
  - Attention: token-sharded fp32 QKV projections, AllToAll to head-sharded
    form (core c: batch c//4, heads (c%4)*4..+4, full causal, fp32),
    AllToAll back, Wo + residual + LN2 + router token-sharded (fp32).
  - MoE: expert-parallel (expert c on core c, bf16 FFN). Dispatch built with
    matmul prefix-sum compaction + indirect DMA; combine via ReduceScatter.

Outputs per core: x [512,1024] f32, probs [512,8] f32, ti [512,2] int32.
"""

import numpy as np
import ml_dtypes

import concourse.bass as bass
import concourse.bacc as bacc
import concourse.mybir as mybir
import concourse.tile as tile
from concourse.bass_utils import run_bass_kernel_spmd

F32 = mybir.dt.float32
F32R = mybir.dt.float32r
BF16 = mybir.dt.bfloat16
I32 = mybir.dt.int32
U32 = mybir.dt.uint32
U8 = mybir.dt.uint8
AF = mybir.ActivationFunctionType
OP = mybir.AluOpType

P = 128
NCORES = 8
B, S, D, H, E, K, F = 2, 2048, 1024, 16, 8, 2, 2048
HD = D // H          # 64
T = B * S            # 4096
TPC = T // NCORES    # 512
TB = 256             # tokens per (core, batch)
DC = D // P          # 8
FC = F // P          # 16
NT = TPC // P        # 4
HU = 4               # head units per core
NQ = S // P          # 16 q-chunks per unit
GCOL = T // P        # 32
CAP = 1152
NA = CAP // P        # 9
NEG = -1.0e30

GROUPS8 = [list(range(NCORES))]

_CACHE = {}


def _build():
    nc = bacc.Bacc("TRN2", target_bir_lowering=False, debug=False,
                   enable_asserts=True, num_devices=NCORES)

    def din(name, shape, dt=F32):
        return nc.dram_tensor(name, shape, dt, kind="ExternalInput")

    io = dict(
        xT_d=din("xT", [D, TPC]),
        wq_d=din("wq", [D, D]), wk_d=din("wk", [D, D]),
        wv_d=din("wv", [D, D]), wo_d=din("wo", [D, D]),
        wr_d=din("wr", [D, E]),
        ln1g_d=din("ln1g", [D, 1]), ln1b_d=din("ln1b", [D, 1]),
        ln2g_d=din("ln2g", [D, 1]), ln2b_d=din("ln2b", [D, 1]),
        bq_d=din("bq", [D, 1]), bk_d=din("bk", [D, 1]), bv_d=din("bv", [D, 1]),
        bo_d=din("bo", [1, D]), br_d=din("br", [1, E]),
        wg_d=din("wg", [D, F], BF16), wu_d=din("wu", [D, F], BF16),
        wd_d=din("wd", [F, D], BF16),
        bg_d=din("bg", [F, 1]), bu_d=din("bu", [F, 1]), bd_d=din("bd", [1, D]),
        cid_d=din("cid", [1, 1]),
        strictu_d=din("strictu", [P, P]),
        negmask_d=din("negmask", [P, P]),
        ident_d=din("ident", [P, P]),
        identbf_d=din("identbf", [P, P], BF16),
        tidf_d=din("tidf", [P, GCOL]),
        xout_d=nc.dram_tensor("xout", [TPC, D], F32, kind="ExternalOutput"),
        probs_d=nc.dram_tensor("probs", [TPC, E], F32, kind="ExternalOutput"),
        ti_d=nc.dram_tensor("ti", [TPC, K], I32, kind="ExternalOutput"),
        a2a1_in=nc.dram_tensor("a2a1_in", [NCORES * 3 * HU * HD, TB], F32),
        a2a1_out=nc.dram_tensor("a2a1_out", [NCORES * 3 * HU * HD, TB], F32),
        a2a2_in=nc.dram_tensor("a2a2_in", [NCORES * HU * HD, TB], F32),
        a2a2_out=nc.dram_tensor("a2a2_out", [NCORES * HU * HD, TB], F32),
        paq_in=nc.dram_tensor("paq_in", [TPC, 4], F32),
        paq_all=nc.dram_tensor("paq_all", [T, 4], F32, addr_space="Shared"),
        xn_in=nc.dram_tensor("xn_in", [TPC, D], BF16),
        xn_all=nc.dram_tensor("xn_all", [T, D], BF16, addr_space="Shared"),
        pairs_dram=nc.dram_tensor("pairs_dram", [CAP + P, 4], F32),
        out_dense=nc.dram_tensor("out_dense", [T + 1, D], BF16),
        rs_out=nc.dram_tensor("rs_outd", [TPC, D], BF16),
    )

    with tile.TileContext(nc) as tc:
        _emit(nc, tc, io)
    nc.compile()
    return nc


def _emit(nc, tc, io):
    g = io
    from contextlib import ExitStack
    ctx = ExitStack()
    with ctx:
        const = ctx.enter_context(tc.tile_pool(name="const", bufs=1))
        p_top = ctx.enter_context(tc.tile_pool(name="p_top", bufs=1))
        p_fin = ctx.enter_context(tc.tile_pool(name="p_fin", bufs=1))

        def rearr_col(d):  # [D,1] / [F,1] -> [P, chunks]
            return d[:, :].rearrange("(c p) o -> p (c o)", p=P)

        ident = const.tile([P, P], F32)
        nc.sync.dma_start(out=ident[:], in_=g["ident_d"][:, :])
        identbf = const.tile([P, P], BF16)
        nc.sync.dma_start(out=identbf[:], in_=g["identbf_d"][:, :])
        negmask = const.tile([P, P], F32)
        nc.sync.dma_start(out=negmask[:], in_=g["negmask_d"][:, :])
        strictu = const.tile([P, P], F32)
        nc.sync.dma_start(out=strictu[:], in_=g["strictu_d"][:, :])
        ones_col = const.tile([P, 1], F32)
        nc.vector.memset(ones_col[:], 1.0)
        ones_r128 = const.tile([1, P], F32)
        nc.vector.memset(ones_r128[:], 1.0)
        ones_r512 = const.tile([1, TPC], F32)
        nc.vector.memset(ones_r512[:], 1.0)
        ln1g = const.tile([P, DC], F32)
        nc.sync.dma_start(out=ln1g[:], in_=rearr_col(g["ln1g_d"]))
        ln1b = const.tile([P, DC], F32)
        nc.sync.dma_start(out=ln1b[:], in_=rearr_col(g["ln1b_d"]))
        ln2g = const.tile([P, DC], F32)
        nc.sync.dma_start(out=ln2g[:], in_=rearr_col(g["ln2g_d"]))
        ln2b = const.tile([P, DC], F32)
        nc.sync.dma_start(out=ln2b[:], in_=rearr_col(g["ln2b_d"]))
        bq_s = const.tile([P, DC], F32)
        nc.sync.dma_start(out=bq_s[:], in_=rearr_col(g["bq_d"]))
        bk_s = const.tile([P, DC], F32)
        nc.sync.dma_start(out=bk_s[:], in_=rearr_col(g["bk_d"]))
        bv_s = const.tile([P, DC], F32)
        nc.sync.dma_start(out=bv_s[:], in_=rearr_col(g["bv_d"]))
        bo_s = const.tile([1, D], F32)
        nc.sync.dma_start(out=bo_s[:], in_=g["bo_d"][:, :])
        br_s = const.tile([1, E], F32)
        nc.sync.dma_start(out=br_s[:], in_=g["br_d"][:, :])
        bg_s = const.tile([P, FC], F32)
        nc.sync.dma_start(out=bg_s[:], in_=rearr_col(g["bg_d"]))
        bu_s = const.tile([P, FC], F32)
        nc.sync.dma_start(out=bu_s[:], in_=rearr_col(g["bu_d"]))
        bd_s = const.tile([1, D], F32)
        nc.sync.dma_start(out=bd_s[:], in_=g["bd_d"][:, :])
        cid_b = const.tile([P, 1], F32)
        nc.sync.dma_start(out=cid_b[:], in_=g["cid_d"][0:1, :].partition_broadcast(P))
        tidf = const.tile([P, GCOL], F32)
        nc.sync.dma_start(out=tidf[:], in_=g["tidf_d"][:, :])
        eps_t = const.tile([1, 1], F32)
        nc.vector.memset(eps_t[:], 1e-5)

        # early zero-fills
        zero_bf = const.tile([P, D], BF16)
        nc.vector.memset(zero_bf[:], 0.0)
        for i in range(T // P):
            nc.sync.dma_start(out=g["out_dense"][i * P:(i + 1) * P, :], in_=zero_bf[:])
        nc.sync.dma_start(out=g["out_dense"][T:T + 1, :], in_=zero_bf[0:1, :])
        zero4 = const.tile([P, 4], F32)
        nc.vector.memset(zero4[:], 0.0)
        for a in range(NA + 1):
            nc.sync.dma_start(out=g["pairs_dram"][a * P:(a + 1) * P, :], in_=zero4[:])

        xT = p_top.tile([P, DC, TPC], F32)
        for dc in range(DC):
            nc.sync.dma_start(out=xT[:, dc, :], in_=g["xT_d"][dc * P:(dc + 1) * P, :])

        # ---------------- LayerNorm helper ----------------
        def layernorm(src, g_ap, b_ap, dst, work, lnps, extra_bf=None):
            sum_ps = lnps.tile([1, TPC], F32, tag="ln_sum")
            ssq_ps = lnps.tile([1, TPC], F32, tag="ln_ssq")
            for dc in range(DC):
                nc.tensor.matmul(out=sum_ps[:], lhsT=ones_col[:], rhs=src[:, dc, :],
                                 start=(dc == 0), stop=(dc == DC - 1))
            for dc in range(DC):
                sq = work.tile([P, TPC], F32, tag="ln_sq")
                nc.vector.tensor_tensor(out=sq[:], in0=src[:, dc, :],
                                        in1=src[:, dc, :], op=OP.mult)
                nc.tensor.matmul(out=ssq_ps[:], lhsT=ones_col[:], rhs=sq[:],
                                 start=(dc == 0), stop=(dc == DC - 1))
            mean = work.tile([1, TPC], F32, tag="ln_mean")
            nc.scalar.mul(out=mean[:], in_=sum_ps[:], mul=1.0 / D)
            msq = work.tile([1, TPC], F32, tag="ln_msq")
            nc.scalar.mul(out=msq[:], in_=ssq_ps[:], mul=1.0 / D)
            var = work.tile([1, TPC], F32, tag="ln_var")
            nc.vector.tensor_tensor(out=var[:], in0=mean[:], in1=mean[:], op=OP.mult)
            nc.vector.tensor_sub(out=var[:], in0=msq[:], in1=var[:])
            std = work.tile([1, TPC], F32, tag="ln_std")
            nc.scalar.activation(out=std[:], in_=var[:], func=AF.Sqrt, bias=eps_t[:])
            rstd = work.tile([1, TPC], F32, tag="ln_rstd")
            nc.vector.reciprocal(out=rstd[:], in_=std[:])
            mean_b_ps = lnps.tile([P, TPC], F32, tag="ln_bc")
            nc.tensor.matmul(out=mean_b_ps[:], lhsT=ones_r128[:], rhs=mean[:],
                             start=True, stop=True)
            mean_b = work.tile([P, TPC], F32, tag="ln_meanb")
            nc.vector.tensor_copy(out=mean_b[:], in_=mean_b_ps[:])
            rstd_b_ps = lnps.tile([P, TPC], F32, tag="ln_bc")
            nc.tensor.matmul(out=rstd_b_ps[:], lhsT=ones_r128[:], rhs=rstd[:],
                             start=True, stop=True)
            rstd_b = work.tile([P, TPC], F32, tag="ln_rstdb")
            nc.vector.tensor_copy(out=rstd_b[:], in_=rstd_b_ps[:])
            for dc in range(DC):
                tmp = work.tile([P, TPC], F32, tag="ln_tmp")
                nc.vector.tensor_sub(out=tmp[:], in0=src[:, dc, :], in1=mean_b[:])
                nc.vector.tensor_tensor(out=tmp[:], in0=tmp[:], in1=rstd_b[:],
                                        op=OP.mult)
                nc.scalar.activation(out=dst[:, dc, :], in_=tmp[:], func=AF.Identity,
                                     bias=b_ap[:, dc:dc + 1], scale=g_ap[:, dc:dc + 1])
                if extra_bf is not None:
                    nc.vector.tensor_copy(out=extra_bf[:, dc, :], in_=dst[:, dc, :])

        stage1 = ctx.enter_context(tc.tile_pool(name="stage1", bufs=1))
        h1n = stage1.tile([P, DC, TPC], F32)
        with tc.tile_pool(name="ln1_work", bufs=3) as ln1_work, \
             tc.tile_pool(name="ln1_ps", bufs=1, space="PSUM") as ln1_ps:
            layernorm(xT, ln1g, ln1b, h1n, ln1_work, ln1_ps)

        # ---------------- QKV projections (Q/K f32r, V fp32) ----------------
        with tc.tile_pool(name="proj", bufs=4) as proj, \
             tc.tile_pool(name="projr", bufs=1) as projr, \
             tc.tile_pool(name="proj_ps", bufs=2, space="PSUM") as proj_ps:
            h1n_r = projr.tile([P, DC, TPC], F32R)
            for dc in range(DC):
                nc.vector.tensor_copy(out=h1n_r[:, dc, :], in_=h1n[:, dc, :])
            for wi, (w_d, b_s, base) in enumerate((
                    (g["wq_d"], bq_s, 0),
                    (g["wk_d"], bk_s, HU * HD),
                    (g["wv_d"], bv_s, 2 * HU * HD))):
                use_r = wi < 2
                for mc in range(DC):
                    acc = proj_ps.tile([P, TPC], F32, tag="proj_acc")
                    for dc in range(DC):
                        wt = proj.tile([P, P], F32, tag="w")
                        nc.sync.dma_start(
                            out=wt[:],
                            in_=w_d[dc * P:(dc + 1) * P, mc * P:(mc + 1) * P])
                        if use_r:
                            wt_r = proj.tile([P, P], F32R, tag="w_r")
                            nc.vector.tensor_copy(out=wt_r[:], in_=wt[:])
                            nc.tensor.matmul(out=acc[:], lhsT=wt_r[:],
                                             rhs=h1n_r[:, dc, :],
                                             start=(dc == 0), stop=(dc == DC - 1))
                        else:
                            nc.tensor.matmul(out=acc[:], lhsT=wt[:],
                                             rhs=h1n[:, dc, :],
                                             start=(dc == 0), stop=(dc == DC - 1))
                    ev = proj.tile([P, TPC], F32, tag="ev")
                    nc.scalar.activation(out=ev[:], in_=acc[:], func=AF.Identity,
                                         bias=b_s[:, mc:mc + 1])
                    # ship: dest d0 = mc//2 (batch0 cols) and d1 = 4 + mc//2
                    half = (mc % 2) * P
                    d0 = mc // 2
                    d1 = 4 + mc // 2
                    nc.sync.dma_start(
                        out=g["a2a1_in"][d0 * 768 + base + half:d0 * 768 + base + half + P, :],
                        in_=ev[:, 0:TB])
                    nc.sync.dma_start(
                        out=g["a2a1_in"][d1 * 768 + base + half:d1 * 768 + base + half + P, :],
                        in_=ev[:, TB:2 * TB])

        nc.gpsimd.collective_compute(
            "AllToAll", OP.bypass, replica_groups=GROUPS8,
            ins=[g["a2a1_in"][:, :]], outs=[g["a2a1_out"][:, :]])

        # ---------------- attention: 4 units, full causal ----------------
        ao_scope = tc.tile_pool(name="p_ao", bufs=1)
        p_ao = ao_scope.__enter__()
        aoTn = p_ao.tile([64, HU, S], F32)
        with tc.tile_pool(name="attn", bufs=2) as attn, \
             tc.tile_pool(name="attn_ps", bufs=1, space="PSUM") as attn_ps, \
             tc.tile_pool(name="attn_ps2", bufs=2, space="PSUM") as attn_ps2:
            for u in range(HU):
                qt = attn.tile([64, S], F32, tag="qt")
                kt = attn.tile([64, S], F32, tag="kt")
                vtok = attn.tile([P, NQ, 64], F32, tag="vtok")
                for i in range(NCORES):
                    r0 = i * 768 + u * 64
                    nc.sync.dma_start(out=qt[:, i * TB:(i + 1) * TB],
                                      in_=g["a2a1_out"][r0:r0 + 64, :])
                    nc.sync.dma_start(out=kt[:, i * TB:(i + 1) * TB],
                                      in_=g["a2a1_out"][r0 + 256:r0 + 256 + 64, :])
                    vt_tmp = attn.tile([64, TB], F32, tag="vt_tmp")
                    nc.sync.dma_start(out=vt_tmp[:],
                                      in_=g["a2a1_out"][r0 + 512:r0 + 512 + 64, :])
                    for j in range(TB // P):
                        vps = attn_ps2.tile([P, 64], F32, tag="pt")
                        nc.tensor.transpose(out=vps[:], in_=vt_tmp[:, j * P:(j + 1) * P],
                                            identity=ident[0:64, 0:64])
                        nc.vector.tensor_copy(out=vtok[:, i * 2 + j, :], in_=vps[:])

                qt_r = attn.tile([64, S], F32R, tag="qt_r")
                nc.vector.tensor_copy(out=qt_r[:], in_=qt[:])
                kt_r = attn.tile([64, S], F32R, tag="kt_r")
                nc.vector.tensor_copy(out=kt_r[:], in_=kt[:])
                for qi in range(NQ):
                    lk = (qi + 1) * P
                    p_sb = attn.tile([P, S], F32, tag="p_sb")
                    rs_tot = attn.tile([P, 1], F32, tag="rs_tot")
                    nhalf = (lk + 1023) // 1024
                    for hh in range(nhalf):
                        h0 = hh * 1024
                        hw = min(1024, lk - h0)
                        sc = attn_ps2.tile([P, 1024], F32, tag="sc")
                        for n0 in range(0, hw, 512):
                            nw = min(512, hw - n0)
                            nc.tensor.matmul(out=sc[:, n0:n0 + nw],
                                             lhsT=qt_r[:, qi * P:(qi + 1) * P],
                                             rhs=kt_r[:, h0 + n0:h0 + n0 + nw],
                                             start=True, stop=True)
                        if h0 + hw == lk:
                            nc.vector.tensor_add(out=sc[:, hw - P:hw],
                                                 in0=sc[:, hw - P:hw], in1=negmask[:])
                        rs_h = attn.tile([P, 1], F32, tag="rs_h")
                        nc.scalar.activation(out=p_sb[:, h0:h0 + hw], in_=sc[:, 0:hw],
                                             func=AF.Exp, scale=1.0 / 8.0,
                                             accum_out=rs_h[:])
                        if hh == 0:
                            nc.vector.tensor_copy(out=rs_tot[:], in_=rs_h[:])
                        else:
                            nc.vector.tensor_add(out=rs_tot[:], in0=rs_tot[:],
                                                 in1=rs_h[:])
                    rec = attn.tile([P, 1], F32, tag="rec")
                    nc.vector.reciprocal(out=rec[:], in_=rs_tot[:])
                    recT_ps = attn_ps.tile([1, P], F32, tag="rec")
                    nc.tensor.transpose(out=recT_ps[:], in_=rec[:], identity=ident[:])
                    recT = attn.tile([1, P], F32, tag="recT_sb")
                    nc.vector.tensor_copy(out=recT[:], in_=recT_ps[:])
                    recb_ps = attn_ps.tile([64, P], F32, tag="rec")
                    nc.tensor.matmul(out=recb_ps[:], lhsT=ones_r128[:, 0:64],
                                     rhs=recT[:], start=True, stop=True)
                    recb = attn.tile([64, P], F32, tag="recb_sb")
                    nc.vector.tensor_copy(out=recb[:], in_=recb_ps[:])
                    ao_ps = attn_ps.tile([64, P], F32, tag="ao")
                    for kc in range(qi + 1):
                        pt_ps = attn_ps2.tile([P, P], F32, tag="pt")
                        nc.tensor.transpose(out=pt_ps[:],
                                            in_=p_sb[:, kc * P:(kc + 1) * P],
                                            identity=ident[:])
                        pt_sb = attn.tile([P, P], F32, tag="pt_sb")
                        nc.vector.tensor_copy(out=pt_sb[:], in_=pt_ps[:])
                        nc.tensor.matmul(out=ao_ps[:], lhsT=vtok[:, kc, :],
                                         rhs=pt_sb[:],
                                         start=(kc == 0), stop=(kc == qi),
                                         skip_group_check=True)
                    nc.vector.tensor_tensor(out=aoTn[:, u, qi * P:(qi + 1) * P],
                                            in0=ao_ps[:], in1=recb[:], op=OP.mult)

        for u in range(HU):
            for d in range(NCORES):
                nc.sync.dma_start(
                    out=g["a2a2_in"][d * 256 + u * 64:d * 256 + (u + 1) * 64, :],
                    in_=aoTn[:, u, d * TB:(d + 1) * TB])
        ao_scope.__exit__(None, None, None)

        nc.gpsimd.collective_compute(
            "AllToAll", OP.bypass, replica_groups=GROUPS8,
            ins=[g["a2a2_in"][:, :]], outs=[g["a2a2_out"][:, :]])

        # ---------------- Wo + residual + LN2 + router ----------------
        xmid_tok = p_fin.tile([P, NT, D], F32)
        wgt_sb = p_fin.tile([P, NA, 4], F32)
        outid_i = p_fin.tile([P, NA], I32)
        gidx_i = p_fin.tile([P, NA], I32)

        with tc.tile_pool(name="p_mid", bufs=1) as p_mid, \
             tc.tile_pool(name="mid", bufs=2) as mid, \
             tc.tile_pool(name="mid_ps", bufs=2, space="PSUM") as mid_ps, \
             tc.tile_pool(name="mid_ps1", bufs=1, space="PSUM") as mid_ps1:
            xmid = p_mid.tile([P, DC, TPC], F32)
            xn_f = p_mid.tile([P, DC, TPC], F32)
            xn_bf = p_mid.tile([P, DC, TPC], BF16)
            for mc in range(DC):
                acc = mid_ps.tile([P, TPC], F32, tag="wo_acc")
                nc.tensor.matmul(out=acc[:], lhsT=bo_s[:, mc * P:(mc + 1) * P],
                                 rhs=ones_r512[:], start=True, stop=False)
                for dc in range(DC):
                    aot = mid.tile([P, TPC], F32, tag="aot")
                    r_b0 = (dc // 2) * 256 + (dc % 2) * P
                    r_b1 = (4 + dc // 2) * 256 + (dc % 2) * P
                    nc.sync.dma_start(out=aot[:, 0:TB],
                                      in_=g["a2a2_out"][r_b0:r_b0 + P, :])
                    nc.sync.dma_start(out=aot[:, TB:2 * TB],
                                      in_=g["a2a2_out"][r_b1:r_b1 + P, :])
                    wt = mid.tile([P, P], F32, tag="wo_w")
                    nc.sync.dma_start(
                        out=wt[:],
                        in_=g["wo_d"][dc * P:(dc + 1) * P, mc * P:(mc + 1) * P])
                    nc.tensor.matmul(out=acc[:], lhsT=wt[:], rhs=aot[:],
                                     start=False, stop=(dc == DC - 1))
                nc.vector.tensor_tensor(out=xmid[:, mc, :], in0=acc[:],
                                        in1=xT[:, mc, :], op=OP.add)

            for mc in range(DC):
                for tcn in range(NT):
                    tp = mid_ps.tile([P, P], F32, tag="tp")
                    nc.tensor.transpose(out=tp[:],
                                        in_=xmid[:, mc, tcn * P:(tcn + 1) * P],
                                        identity=ident[:])
                    nc.vector.tensor_copy(out=xmid_tok[:, tcn, mc * P:(mc + 1) * P],
                                          in_=tp[:])

            layernorm(xmid, ln2g, ln2b, xn_f, mid, mid_ps1, extra_bf=xn_bf)

            lg_ps = mid_ps1.tile([E, TPC], F32, tag="lg")
            nc.tensor.matmul(out=lg_ps[:], lhsT=br_s[:], rhs=ones_r512[:],
                             start=True, stop=False)
            for dc in range(DC):
                wrt = mid.tile([P, E], F32, tag="wr")
                nc.sync.dma_start(out=wrt[:], in_=g["wr_d"][dc * P:(dc + 1) * P, :])
                nc.tensor.matmul(out=lg_ps[:], lhsT=wrt[:], rhs=xn_f[:, dc, :],
                                 start=False, stop=(dc == DC - 1))
            lg_sb = mid.tile([E, TPC], F32, tag="lg_sb")
            nc.vector.tensor_copy(out=lg_sb[:], in_=lg_ps[:])

            paq_sb = mid.tile([P, NT, 4], F32, tag="paq")
            for tcn in range(NT):
                lt_ps = mid_ps.tile([P, P], F32, tag="tp")
                nc.tensor.transpose(out=lt_ps[:, 0:E],
                                    in_=lg_sb[:, tcn * P:(tcn + 1) * P],
                                    identity=ident[0:E, 0:E])
                lt = mid.tile([P, E], F32, tag="lt_sb")
                nc.vector.tensor_copy(out=lt[:], in_=lt_ps[:, 0:E])
                mx = mid.tile([P, 8], F32, tag="mx")
                mi = mid.tile([P, 8], U32, tag="mi")
                nc.vector.max_with_indices(mx[:], mi[:], lt[:])
                ti_sb = mid.tile([P, K], I32, tag="ti_sb")
                nc.vector.tensor_copy(out=ti_sb[:], in_=mi[:, 0:K].bitcast(I32))
                nc.sync.dma_start(out=g["ti_d"][tcn * P:(tcn + 1) * P, :], in_=ti_sb[:])
                negmx = mid.tile([P, 1], F32, tag="negmx")
                nc.vector.tensor_scalar_mul(negmx[:], mx[:, 0:1], -1.0)
                e8 = mid.tile([P, E], F32, tag="e8")
                s8 = mid.tile([P, 1], F32, tag="s8")
                nc.scalar.activation(out=e8[:], in_=lt[:], func=AF.Exp,
                                     bias=negmx[:], accum_out=s8[:])
                r8 = mid.tile([P, 1], F32, tag="r8")
                nc.vector.reciprocal(out=r8[:], in_=s8[:])
                pr = mid.tile([P, E], F32, tag="pr")
                nc.vector.tensor_scalar(out=pr[:], in0=e8[:], scalar1=r8[:],
                                        scalar2=None, op0=OP.mult)
                nc.sync.dma_start(out=g["probs_d"][tcn * P:(tcn + 1) * P, :], in_=pr[:])
                dif = mid.tile([P, 1], F32, tag="dif")
                nc.vector.tensor_sub(out=dif[:], in0=mx[:, 0:1], in1=mx[:, 1:2])
                tp0 = mid.tile([P, 1], F32, tag="tp0")
                nc.scalar.activation(out=tp0[:], in_=dif[:], func=AF.Sigmoid)
                nc.vector.tensor_copy(out=paq_sb[:, tcn, 0:1], in_=tp0[:])
                nc.vector.tensor_scalar(out=paq_sb[:, tcn, 1:2], in0=tp0[:],
                                        scalar1=-1.0, scalar2=1.0,
                                        op0=OP.mult, op1=OP.add)
                nc.vector.tensor_copy(out=paq_sb[:, tcn, 2:3], in_=mi[:, 0:1])
                nc.vector.tensor_copy(out=paq_sb[:, tcn, 3:4], in_=mi[:, 1:2])
            nc.sync.dma_start(
                out=g["paq_in"][:, :].rearrange("(c p) k -> p c k", p=P),
                in_=paq_sb[:])

            xn_tok = mid.tile([P, NT, D], BF16, tag="xn_tok")
            for dc in range(DC):
                for tcn in range(NT):
                    tp = mid_ps.tile([P, P], BF16, tag="tp")
                    nc.tensor.transpose(out=tp[:],
                                        in_=xn_bf[:, dc, tcn * P:(tcn + 1) * P],
                                        identity=identbf[:])
                    nc.vector.tensor_copy(out=xn_tok[:, tcn, dc * P:(dc + 1) * P],
                                          in_=tp[:])
            for tcn in range(NT):
                nc.sync.dma_start(out=g["xn_in"][tcn * P:(tcn + 1) * P, :],
                                  in_=xn_tok[:, tcn, :])

        nc.gpsimd.collective_compute(
            "AllGather", OP.bypass, replica_groups=GROUPS8,
            ins=[g["paq_in"][:, :]], outs=[g["paq_all"][:, :]])
        nc.gpsimd.collective_compute(
            "AllGather", OP.bypass, replica_groups=GROUPS8,
            ins=[g["xn_in"][:, :]], outs=[g["xn_all"][:, :]])

        # ---------------- dispatch ----------------
        with tc.tile_pool(name="disp", bufs=2) as disp, \
             tc.tile_pool(name="disp_ps", bufs=2, space="PSUM") as disp_ps:
            paq_t = disp.tile([P, GCOL, 4], F32, tag="paq_t")
            nc.sync.dma_start(out=paq_t[:],
                              in_=g["paq_all"][:, :].rearrange("(c p) k -> p c k", p=P))
            m0 = disp.tile([P, GCOL], F32, tag="m0")
            nc.vector.tensor_tensor(out=m0[:], in0=paq_t[:, :, 2],
                                    in1=cid_b[:].to_broadcast([P, GCOL]),
                                    op=OP.is_equal)
            m1 = disp.tile([P, GCOL], F32, tag="m1")
            nc.vector.tensor_tensor(out=m1[:], in0=paq_t[:, :, 3],
                                    in1=cid_b[:].to_broadcast([P, GCOL]),
                                    op=OP.is_equal)
            maskf = disp.tile([P, GCOL], F32, tag="maskf")
            nc.vector.tensor_add(out=maskf[:], in0=m0[:], in1=m1[:])
            mask8 = disp.tile([P, GCOL], U8, tag="mask8")
            nc.vector.tensor_copy(out=mask8[:], in_=maskf[:])
            wsel = disp.tile([P, GCOL], F32, tag="wsel")
            nc.vector.tensor_tensor(out=m0[:], in0=m0[:], in1=paq_t[:, :, 0], op=OP.mult)
            nc.vector.tensor_tensor(out=m1[:], in0=m1[:], in1=paq_t[:, :, 1], op=OP.mult)
            nc.vector.tensor_add(out=wsel[:], in0=m0[:], in1=m1[:])

            cs_ps = disp_ps.tile([1, GCOL], F32, tag="cs")
            nc.tensor.matmul(out=cs_ps[:], lhsT=ones_col[:], rhs=maskf[:],
                             start=True, stop=True)
            pfx_a = disp.tile([1, GCOL], F32, tag="pfx_a")
            pfx_b = disp.tile([1, GCOL], F32, tag="pfx_b")
            nc.vector.memset(pfx_a[:], 0.0)
            nc.vector.tensor_copy(out=pfx_a[:, 1:GCOL], in_=cs_ps[0:1, 0:GCOL - 1])
            cur, oth = pfx_a, pfx_b
            for sh in (1, 2, 4, 8, 16):
                nc.vector.tensor_copy(out=oth[:], in_=cur[:])
                nc.vector.tensor_add(out=oth[:, sh:GCOL], in0=cur[:, sh:GCOL],
                                     in1=cur[:, 0:GCOL - sh])
                cur, oth = oth, cur
            p_ps = disp_ps.tile([P, GCOL], F32, tag="p_ps")
            nc.tensor.matmul(out=p_ps[:], lhsT=strictu[:], rhs=maskf[:],
                             start=True, stop=False)
            nc.tensor.matmul(out=p_ps[:], lhsT=ones_r128[:], rhs=cur[:],
                             start=False, stop=True)
            trash = disp.tile([P, GCOL], F32, tag="trash")
            nc.vector.memset(trash[:], float(CAP))
            p_sb2 = disp.tile([P, GCOL], F32, tag="p_sb2")
            nc.vector.tensor_copy(out=p_sb2[:], in_=p_ps[:])
            pos_f = disp.tile([P, GCOL], F32, tag="pos_f")
            nc.vector.select(out=pos_f[:], mask=mask8[:], on_true=p_sb2[:],
                             on_false=trash[:])
            nc.vector.tensor_scalar_min(pos_f[:], pos_f[:], float(CAP))
            pos_i = disp.tile([P, GCOL], I32, tag="pos_i")
            nc.vector.tensor_copy(out=pos_i[:], in_=pos_f[:])

            vals4 = disp.tile([P, GCOL, 4], F32, tag="vals4")
            nc.vector.tensor_copy(out=vals4[:, :, 0], in_=tidf[:])
            nc.vector.tensor_copy(out=vals4[:, :, 1], in_=wsel[:])
            nc.vector.memset(vals4[:, :, 2], 1.0)
            nc.vector.memset(vals4[:, :, 3], 0.0)
            for col in range(GCOL):
                nc.gpsimd.indirect_dma_start(
                    out=g["pairs_dram"][:, :],
                    out_offset=bass.IndirectOffsetOnAxis(
                        ap=pos_i[:, col:col + 1], axis=0),
                    in_=vals4[:, col, :], in_offset=None)

            nc.sync.dma_start(
                out=wgt_sb[:],
                in_=g["pairs_dram"][0:CAP, :].rearrange("(a p) k -> p a k", p=P))
            nc.vector.tensor_copy(out=gidx_i[:], in_=wgt_sb[:, :, 0])
            valid8 = disp.tile([P, NA], U8, tag="valid8")
            nc.vector.tensor_copy(out=valid8[:], in_=wgt_sb[:, :, 2])
            t4096 = disp.tile([P, NA], F32, tag="t4096")
            nc.vector.memset(t4096[:], float(T))
            outid_f = disp.tile([P, NA], F32, tag="outid_f")
            nc.vector.select(out=outid_f[:], mask=valid8[:], on_true=wgt_sb[:, :, 0],
                             on_false=t4096[:])
            nc.vector.tensor_copy(out=outid_i[:], in_=outid_f[:])

        # ---------------- expert FFN (bf16) ----------------
        with tc.tile_pool(name="ffn", bufs=2) as ffn, \
             tc.tile_pool(name="ffn_w", bufs=2) as ffn_w, \
             tc.tile_pool(name="ffn_big", bufs=1) as ffn_big, \
             tc.tile_pool(name="ffn_ps", bufs=2, space="PSUM") as ffn_ps:
            xgT = ffn_big.tile([P, DC, CAP], BF16)
            with tc.tile_pool(name="xg_pool", bufs=1) as xgp:
                xg_tok = xgp.tile([P, NA, D], BF16)
                for a in range(NA):
                    nc.gpsimd.indirect_dma_start(
                        out=xg_tok[:, a, :], out_offset=None,
                        in_=g["xn_all"][:, :],
                        in_offset=bass.IndirectOffsetOnAxis(ap=gidx_i[:, a:a + 1], axis=0))
                for a in range(NA):
                    for dc in range(DC):
                        tp = ffn_ps.tile([P, P], BF16, tag="xg_t")
                        nc.tensor.transpose(out=tp[:],
                                            in_=xg_tok[:, a, dc * P:(dc + 1) * P],
                                            identity=identbf[:])
                        nc.vector.tensor_copy(out=xgT[:, dc, a * P:(a + 1) * P], in_=tp[:])

            hg = ffn_big.tile([P, FC, CAP], BF16)
            NCH = tuple((n0, min(512, CAP - n0)) for n0 in range(0, CAP, 512))
            for fc in range(FC):
                wgt_f = ffn_w.tile([P, DC, P], BF16, tag="wg_f")
                nc.sync.dma_start(
                    out=wgt_f[:],
                    in_=g["wg_d"][:, fc * P:(fc + 1) * P].rearrange(
                        "(dc p) f -> p dc f", p=P))
                wut_f = ffn_w.tile([P, DC, P], BF16, tag="wu_f")
                nc.sync.dma_start(
                    out=wut_f[:],
                    in_=g["wu_d"][:, fc * P:(fc + 1) * P].rearrange(
                        "(dc p) f -> p dc f", p=P))
                for (n0, nw) in NCH:
                    g_ps = ffn_ps.tile([P, 512], F32, tag="g_ps")
                    u_ps = ffn_ps.tile([P, 512], F32, tag="u_ps")
                    for dc in range(DC):
                        nc.tensor.matmul(out=g_ps[:, 0:nw], lhsT=wgt_f[:, dc, :],
                                         rhs=xgT[:, dc, n0:n0 + nw],
                                         start=(dc == 0), stop=(dc == DC - 1))
                    for dc in range(DC):
                        nc.tensor.matmul(out=u_ps[:, 0:nw], lhsT=wut_f[:, dc, :],
                                         rhs=xgT[:, dc, n0:n0 + nw],
                                         start=(dc == 0), stop=(dc == DC - 1))
                    g_sb = ffn.tile([P, 512], BF16, tag="g_sb")
                    nc.scalar.activation(out=g_sb[:, 0:nw], in_=g_ps[:, 0:nw],
                                         func=AF.Silu, bias=bg_s[:, fc:fc + 1])
                    nc.vector.scalar_tensor_tensor(
                        out=hg[:, fc, n0:n0 + nw], in0=u_ps[:, 0:nw],
                        scalar=bu_s[:, fc:fc + 1], in1=g_sb[:, 0:nw],
                        op0=OP.add, op1=OP.mult)

            wd_sb = ffn_big.tile([P, FC, D], BF16)
            nc.sync.dma_start(out=wd_sb[:],
                              in_=g["wd_d"][:, :].rearrange("(fc p) d -> p fc d", p=P))
            ffn_out = ffn_big.tile([P, NA, D], BF16)
            for a in range(NA):
                for (n0, nw) in ((0, 512), (512, 512)):
                    o_ps = ffn_ps.tile([P, 512], F32, tag="o_ps")
                    nc.tensor.matmul(out=o_ps[:, 0:nw], lhsT=ones_r128[:],
                                     rhs=bd_s[:, n0:n0 + nw], start=True, stop=False)
                    for fc in range(FC):
                        nc.tensor.matmul(out=o_ps[:, 0:nw],
                                         lhsT=hg[:, fc, a * P:(a + 1) * P],
                                         rhs=wd_sb[:, fc, n0:n0 + nw],
                                         start=False, stop=(fc == FC - 1))
                    nc.scalar.mul(out=ffn_out[:, a, n0:n0 + nw], in_=o_ps[:, 0:nw],
                                  mul=wgt_sb[:, a, 1:2])
            for a in range(NA):
                nc.gpsimd.indirect_dma_start(
                    out=g["out_dense"][:, :],
                    out_offset=bass.IndirectOffsetOnAxis(
                        ap=outid_i[:, a:a + 1], axis=0),
                    in_=ffn_out[:, a, :], in_offset=None)

        nc.gpsimd.collective_compute(
            "ReduceScatter", OP.add, replica_groups=GROUPS8,
            ins=[g["out_dense"][0:T, :]], outs=[g["rs_out"][:, :]])
        with tc.tile_pool(name="fin", bufs=2) as fin:
            for tcn in range(NT):
                rs_sb = fin.tile([P, D], BF16, tag="rs_sb")
                nc.sync.dma_start(out=rs_sb[:],
                                  in_=g["rs_out"][tcn * P:(tcn + 1) * P, :])
                xo = fin.tile([P, D], F32, tag="xo")
                nc.vector.tensor_tensor(out=xo[:], in0=xmid_tok[:, tcn, :],
                                        in1=rs_sb[:], op=OP.add)
                nc.sync.dma_start(out=g["xout_d"][tcn * P:(tcn + 1) * P, :], in_=xo[:])


def _vt2flat():
    """vt -> row index into x.reshape(T, D); vt = c*512 + j."""
    idx = np.zeros(T, np.int64)
    for c in range(NCORES):
        j = np.arange(TPC)
        b = (j >= TB).astype(np.int64)
        s = c * TB + np.where(j < TB, j, j - TB)
        idx[c * TPC:(c + 1) * TPC] = b * S + s
    return idx


def _prep_inputs(inputs):
    f32 = lambda a: np.ascontiguousarray(np.asarray(a, dtype=np.float32))
    bf = lambda a: np.ascontiguousarray(
        np.asarray(a, dtype=np.float32).astype(ml_dtypes.bfloat16))

    x = f32(inputs["x"])
    flat = x.reshape(T, D)
    vt = _vt2flat()

    Wg = np.asarray(inputs["Wg"], dtype=np.float32)
    Wu = np.asarray(inputs["Wu"], dtype=np.float32)
    Wd = np.asarray(inputs["Wd"], dtype=np.float32)
    bg = np.asarray(inputs["bg"], dtype=np.float32)
    bu = np.asarray(inputs["bu"], dtype=np.float32)
    bd = np.asarray(inputs["bd"], dtype=np.float32)

    strictu = np.triu(np.ones((P, P), np.float32), 1)
    negmask = np.triu(np.full((P, P), NEG, np.float32), 1)
    ident = np.eye(P, dtype=np.float32)
    identbf = np.eye(P, dtype=np.float32).astype(ml_dtypes.bfloat16)
    tidf = (np.arange(GCOL)[None, :] * P + np.arange(P)[:, None]).astype(np.float32)

    common = dict(
        wq=f32(inputs["Wq"]), wk=f32(inputs["Wk"]), wv=f32(inputs["Wv"]),
        wo=f32(inputs["Wo"]), wr=f32(inputs["Wr"]),
        ln1g=f32(inputs["ln1_g"]).reshape(D, 1),
        ln1b=f32(inputs["ln1_b"]).reshape(D, 1),
        ln2g=f32(inputs["ln2_g"]).reshape(D, 1),
        ln2b=f32(inputs["ln2_b"]).reshape(D, 1),
        bq=f32(inputs["bq"]).reshape(D, 1), bk=f32(inputs["bk"]).reshape(D, 1),
        bv=f32(inputs["bv"]).reshape(D, 1), bo=f32(inputs["bo"]).reshape(1, D),
        br=f32(inputs["br"]).reshape(1, E),
        strictu=strictu, negmask=negmask, ident=ident, identbf=identbf,
        tidf=tidf,
    )
    in_maps = []
    for c in range(NCORES):
        m = dict(common)
        m["xT"] = np.ascontiguousarray(flat[vt[c * TPC:(c + 1) * TPC]].T)
        m["wg"] = bf(Wg[c])
        m["wu"] = bf(Wu[c])
        m["wd"] = bf(Wd[c])
        m["bg"] = f32(bg[c]).reshape(F, 1)
        m["bu"] = f32(bu[c]).reshape(F, 1)
        m["bd"] = f32(bd[c]).reshape(1, D)
        m["cid"] = np.array([[float(c)]], np.float32)
        in_maps.append(m)
    return in_maps


def kernel(**inputs):
    if "nc" not in _CACHE:
        _CACHE["nc"] = _build()
    nc = _CACHE["nc"]
    in_maps = _prep_inputs(inputs)
    res = run_bass_kernel_spmd(nc, in_maps, list(range(NCORES)))
    vt = _vt2flat()
    x = np.zeros((T, D), np.float32)
    probs = np.zeros((T, E), np.float32)
    ti = np.zeros((T, K), np.int32)
    for c in range(NCORES):
        r = res.results[c]
        sl = vt[c * TPC:(c + 1) * TPC]
        x[sl] = r["xout"]
        probs[sl] = r["probs"]
        ti[sl] = r["ti"]
    return (x.reshape(B, S, D), probs.reshape(B, S, E), ti.reshape(B, S, K))
